# revision 3
# baseline (speedup 1.0000x reference)
"""Trainium2 Bass kernel for nn_KineticModel (gnn_message_passing).

Math (from the reference):
    conc    = scatter(conc_balanced, exp(log_conc_unbalanced))      # [S]
    logc    = log(conc)                                             # [S]
    logv    = log_kcat + relu(-S_mat).T @ logc                      # [R]
    v       = exp(logv)
    dcdt    = (S_mat @ v)[balanced_species]                         # [7680]

Shapes: S_mat [8192, 16384] f32, 8 cores, sharded along the reaction axis
(2048 reactions per core).

Design notes:
  * S_mat entries are small integers in {-2..2} -> exact in bf16, halving
    HBM traffic.  Each core reads its 2048-reaction slice twice, in two
    layouts (species-major tiles for the flux matvec, reaction-major tiles
    for S@v), because the TensorEngine always contracts over the partition
    axis.  Total DMA: 64 MB/core ~= the f32-read-once roofline.
  * f32 vectors are fed to the bf16 matmuls as hi/lo bf16 pairs
    (x = hi + lo, each exactly representable); PSUM accumulates in f32,
    so precision stays ~f32 (rel err ~1e-5).
  * matvec1 (logv): lhsT = relu(-S) tile [128 sp, 128 rxn], rhs =
    [logc_hi, logc_lo] [128, 2]; accumulate over 64 species blocks into
    one PSUM bank [128, 32] (16 rxn blocks x hi/lo interleaved groups).
  * matvec2 (S@v): lhsT = [v_hi, v_lo] [128 rxn, 2] (tiny weight load!),
    rhs = S^T tile [128 rxn, 512 sp] streams at N=512; accumulate over the
    16 rxn blocks into PSUM [2, 2048] per species quarter.  The [2, 8192]
    (hi,lo) partial rows are summed on the host during unsharding.
"""

import sys

if "/opt/trn_rl_repo" not in sys.path:
    sys.path.insert(0, "/opt/trn_rl_repo")

import numpy as np
import ml_dtypes

import concourse.bacc as bacc
import concourse.mybir as mybir
from concourse.tile import TileContext
from concourse.bass_utils import run_bass_kernel_spmd

F32 = mybir.dt.float32
BF16 = mybir.dt.bfloat16
BF16_NP = ml_dtypes.bfloat16

N_SPECIES = 8192
N_RXN = 16384
N_BAL = 7680
N_CORES = 8
R_CORE = N_RXN // N_CORES        # 2048 reactions per core
SB = N_SPECIES // 128            # 64 species blocks
RB = R_CORE // 128               # 16 reaction blocks per core
NQ = 4                           # species quarters for matvec2 psum
QS = N_SPECIES // NQ             # 2048 species per quarter
NCHUNK = QS // 512               # 4 N=512 chunks per quarter

_CACHE = {}


def _build_nc():
    nc = bacc.Bacc(None, target_bir_lowering=False, debug=False)
    s_a = nc.declare_dram_parameter("s_a", [SB, 128, R_CORE], BF16, isOutput=False)
    s_b = nc.declare_dram_parameter("s_b", [NQ, RB, 128, QS], BF16, isOutput=False)
    xa = nc.declare_dram_parameter("xa", [128, SB], F32, isOutput=False)
    xb = nc.declare_dram_parameter("xb", [128, SB], F32, isOutput=False)
    kcat = nc.declare_dram_parameter("kcat", [128, RB], F32, isOutput=False)
    out = nc.declare_dram_parameter("out", [2, N_SPECIES], F32, isOutput=True)

    ts = mybir.AluOpType
    with TileContext(nc) as tc:
        with (
            tc.tile_pool(name="small", bufs=1) as small,
            tc.tile_pool(name="sa", bufs=6) as sa_pool,
            tc.tile_pool(name="rl", bufs=4) as rl_pool,
            tc.tile_pool(name="sb", bufs=8) as sb_pool,
            tc.tile_pool(name="stage", bufs=3) as stage_pool,
            tc.tile_pool(name="psv", bufs=1, space="PSUM") as psv_pool,
            tc.tile_pool(name="psd", bufs=1, space="PSUM") as psd_pool,
        ):
            # ---- logc = Ln(xa) + xb, split into interleaved hi/lo bf16 ----
            xa_t = small.tile([128, SB], F32, tag="xa")
            xb_t = small.tile([128, SB], F32, tag="xb")
            kcat_t = small.tile([128, RB], F32, tag="kcat")
            nc.sync.dma_start(out=xa_t, in_=xa[:])
            nc.sync.dma_start(out=xb_t, in_=xb[:])
            nc.sync.dma_start(out=kcat_t, in_=kcat[:])

            lg = small.tile([128, SB], F32, tag="lg")
            nc.scalar.activation(lg, xa_t, mybir.ActivationFunctionType.Ln)
            logc = small.tile([128, SB], F32, tag="logc")
            nc.vector.tensor_tensor(out=logc, in0=lg, in1=xb_t, op=ts.add)

            logc_hl = small.tile([128, 2 * SB], BF16, tag="logc_hl")
            nc.vector.tensor_copy(out=logc_hl[:, 0 : 2 * SB : 2], in_=logc)
            lh_f = small.tile([128, SB], F32, tag="lh_f")
            nc.vector.tensor_copy(out=lh_f, in_=logc_hl[:, 0 : 2 * SB : 2])
            nc.vector.tensor_tensor(
                out=logc_hl[:, 1 : 2 * SB : 2], in0=logc, in1=lh_f, op=ts.subtract
            )

            # ---- matvec1: psum_v[:, 2*rb:2*rb+2] += relu(-S_A[sb]).T @ logc_hl ----
            psum_v = psv_pool.tile([128, 2 * RB], F32, tag="psum_v")
            for sb in range(SB):
                at = sa_pool.tile([128, R_CORE], BF16, tag="sa")
                nc.sync.dma_start(out=at, in_=s_a[sb])
                rt = rl_pool.tile([128, R_CORE], BF16, tag="rl")
                # relu(-x) = min(x, 0) * -1
                nc.vector.tensor_scalar(
                    out=rt, in0=at, scalar1=0.0, scalar2=-1.0, op0=ts.min, op1=ts.mult
                )
                for rb in range(RB):
                    # start=True clears has_written for the WHOLE bank, so only
                    # the very first matmul may set it; later rb regions' first
                    # writes land on cleared bits and store (not accumulate).
                    nc.tensor.matmul(
                        psum_v[:, 2 * rb : 2 * rb + 2],
                        rt[:, rb * 128 : (rb + 1) * 128],
                        logc_hl[:, 2 * sb : 2 * sb + 2],
                        start=(sb == 0 and rb == 0),
                        stop=(sb == SB - 1 and rb == RB - 1),
                        skip_group_check=True,
                    )

            # ---- v = exp(psum_even + psum_odd + kcat), hi/lo split ----
            pv_sb = small.tile([128, 2 * RB], F32, tag="pv_sb")
            nc.vector.tensor_copy(out=pv_sb, in_=psum_v)
            lv = small.tile([128, RB], F32, tag="lv")
            nc.vector.tensor_tensor(
                out=lv, in0=pv_sb[:, 0 : 2 * RB : 2], in1=pv_sb[:, 1 : 2 * RB : 2],
                op=ts.add,
            )
            lvk = small.tile([128, RB], F32, tag="lvk")
            nc.vector.tensor_tensor(out=lvk, in0=lv, in1=kcat_t, op=ts.add)
            v_f = small.tile([128, RB], F32, tag="v_f")
            nc.scalar.activation(v_f, lvk, mybir.ActivationFunctionType.Exp)

            v_hl = small.tile([128, 2 * RB], BF16, tag="v_hl")
            nc.vector.tensor_copy(out=v_hl[:, 0 : 2 * RB : 2], in_=v_f)
            vh_f = small.tile([128, RB], F32, tag="vh_f")
            nc.vector.tensor_copy(out=vh_f, in_=v_hl[:, 0 : 2 * RB : 2])
            nc.vector.tensor_tensor(
                out=v_hl[:, 1 : 2 * RB : 2], in0=v_f, in1=vh_f, op=ts.subtract
            )

            # ---- matvec2: psum_dc[:, c*512:...] += v_hl[rb].T @ S_B[q, rb][:, c] ----
            for q in range(NQ):
                psum_dc = psd_pool.tile([2, QS], F32, tag="psum_dc")
                for rb in range(RB):
                    bt = sb_pool.tile([128, QS], BF16, tag="sb")
                    nc.sync.dma_start(out=bt, in_=s_b[q, rb])
                    for c in range(NCHUNK):
                        nc.tensor.matmul(
                            psum_dc[:, c * 512 : (c + 1) * 512],
                            v_hl[:, 2 * rb : 2 * rb + 2],
                            bt[:, c * 512 : (c + 1) * 512],
                            start=(rb == 0),
                            stop=(rb == RB - 1),
                        )
                st = stage_pool.tile([2, QS], F32, tag="stage")
                nc.vector.tensor_copy(out=st, in_=psum_dc)
                nc.sync.dma_start(out=out[:, q * QS : (q + 1) * QS], in_=st)
    nc.compile()
    return nc


def _prep_inputs(conc_balanced, S, balanced_species, unbalanced_species,
                 log_conc_unbalanced, log_kcat):
    """Host-side shard + layout prep (pure data movement / dtype casts)."""
    in_maps = []
    # xa: Ln input (1.0 on unbalanced lanes), xb: additive log-term
    xa_full = np.ones(N_SPECIES, dtype=np.float32)
    xb_full = np.zeros(N_SPECIES, dtype=np.float32)
    xa_full[np.asarray(balanced_species)] = np.asarray(conc_balanced)
    xb_full[np.asarray(unbalanced_species)] = np.asarray(log_conc_unbalanced)
    xa_pm = np.ascontiguousarray(xa_full.reshape(SB, 128).T)
    xb_pm = np.ascontiguousarray(xb_full.reshape(SB, 128).T)

    S = np.asarray(S)
    log_kcat = np.asarray(log_kcat)
    for c in range(N_CORES):
        r0 = c * R_CORE
        sl = S[:, r0 : r0 + R_CORE].astype(BF16_NP)          # [8192, 2048]
        s_a = np.ascontiguousarray(sl.reshape(SB, 128, R_CORE))
        # s_b[q, rb, p, j] = S[q*QS + j, r0 + rb*128 + p]
        s_b = np.ascontiguousarray(
            sl.reshape(NQ, QS, RB, 128).transpose(0, 2, 3, 1)
        )
        kcat_pm = np.ascontiguousarray(
            log_kcat[r0 : r0 + R_CORE].astype(np.float32).reshape(RB, 128).T
        )
        in_maps.append(
            {"s_a": s_a, "s_b": s_b, "xa": xa_pm, "xb": xb_pm, "kcat": kcat_pm}
        )
    return in_maps


def kernel(**inputs) -> np.ndarray:
    if "nc" not in _CACHE:
        _CACHE["nc"] = _build_nc()
    nc = _CACHE["nc"]
    in_maps = _prep_inputs(**inputs)
    res = run_bass_kernel_spmd(nc, in_maps, core_ids=list(range(N_CORES)))
    acc = np.zeros(N_SPECIES, dtype=np.float64)
    for c in range(N_CORES):
        o = res.results[c]["out"]
        acc += o[0].astype(np.float64) + o[1].astype(np.float64)
    return acc[:N_BAL].astype(np.float32)
